# revision 1
# baseline (speedup 1.0000x reference)
"""GAT layer (nn_GAT_layer) Trainium2 Bass kernel — 8-core SPMD, row-sharded.

Strategy (per core c of 8, query rows R_c = c*1024 .. (c+1)*1024):
  - Math rewrite: with x_ij = s1_i + s2_j + a_b and leaky(x) = 0.2x + 0.8*relu(x),
      exp(leaky(x)) = exp(0.2*(s1_i+a_b)) * exp(0.2*s2_j + 0.8*relu(x_ij))
    The first factor is constant per row i and cancels in the softmax, so the
    effective unnormalized weight is
      z_ij = mask_ij * exp(0.8*relu(x_ij) + 0.2*s2_j - C)
    (C = 5 is a global shift that also cancels; it keeps exp() within fp16 range.)
    No row-max subtraction is needed (logits are bounded).
  - Work in the transposed layout (key nodes j on partitions): row-sums and
    attn @ h_hat both come out of PE matmuls with stationary [h_hat_j | ones].
  - The mask row-block is loaded *pre-transposed* by the DMA xbar: the int32
    mask is viewed as uint16 pairs and only the low halves (value 0/1) are
    gathered with stride 2, transposed into [128 j, 1024 i] tiles.
  - h_hat ([N,64]) is computed redundantly on every core from h via PE
    transposes; the per-core row slice additionally yields s1.

Self-contained: hardcodes shapes from the problem spec; no sibling imports.
"""

import os
import sys

import numpy as np

for _p in ("/opt/trn_rl_repo", "/root/.axon_site/_ro/trn_rl_repo"):
    if os.path.isdir(_p) and _p not in sys.path:
        sys.path.insert(0, _p)

import concourse.bass as bass
import concourse.bacc as bacc
import concourse.tile as tile
from concourse import mybir
from concourse.masks import make_identity
from concourse.bass_utils import run_bass_kernel_spmd

N, FIN, FOUT, CORES = 8192, 256, 64, 8
P = 128
RPC = N // CORES            # 1024 query rows per core
NJT = N // P                # 64 key tiles (j on partitions)
NHT = N // P                # 64 h row-tiles
NIB = RPC // P              # 8 output row-blocks per core
KC = FIN // P               # 2 contraction chunks for h_hat
HALF = RPC // 2             # 512: matmul moving-dim max
C_SHIFT = 5.0               # global logit shift (cancels in softmax)

f32 = mybir.dt.float32
f16 = mybir.dt.float16
i32 = mybir.dt.int32
AF = mybir.ActivationFunctionType
OP = mybir.AluOpType


def _dummy_out(nc, tc, out_d):
    with tc.tile_pool(name="dummy", bufs=1) as dp:
        for ib in range(NIB):
            t = dp.tile([P, FOUT], f32, tag="d")
            nc.vector.memset(t, 0.0)
            nc.sync.dma_start(out=out_d[ib * P:(ib + 1) * P, :], in_=t)


def build_nc(reps: int = 1, debug: bool = False, stage: int = 99,
             timing: bool = False, dyn_reps: int = 0) -> bass.Bass:
    """stage: 1=params only, 2=+h_hat, 3=+s2/s1, 4=+main loop, 99=full.
    timing: declare tiny h/mask inputs and read them repeatedly at offset 0 —
    identical on-device work, tiny host->device transfer (for wall timing)."""
    nc = bacc.Bacc(None)

    h_full = nc.dram_tensor("h_full", [P if timing else N, FIN], f32,
                            kind="ExternalInput")[:]
    h_rows = nc.dram_tensor("h_rows", [P if timing else RPC, FIN], f32,
                            kind="ExternalInput")[:]
    mask_t = nc.dram_tensor("maskT_rows", [P if timing else N, RPC], i32,
                            kind="ExternalInput")[:]

    def hs(i):
        return 0 if timing else i
    w_w = nc.dram_tensor("W_w", [FOUT, FIN], f32, kind="ExternalInput")[:]
    w_b = nc.dram_tensor("W_b_row", [1, FOUT], f32, kind="ExternalInput")[:]
    a1_d = nc.dram_tensor("a1_col", [FOUT, 1], f32, kind="ExternalInput")[:]
    a2_d = nc.dram_tensor("a2_row", [1, FOUT], f32, kind="ExternalInput")[:]
    ab_d = nc.dram_tensor("a_b_s", [1, 1], f32, kind="ExternalInput")[:]
    out_d = nc.dram_tensor("out_rows", [RPC, FOUT], f32, kind="ExternalOutput")[:]

    with tile.TileContext(nc) as tc:
        with tc.tile_pool(name="consts", bufs=1) as consts:
            ident = consts.tile([P, P], f32)
            make_identity(nc, ident)
            ident16 = consts.tile([P, P], f16)
            make_identity(nc, ident16)
            ones1 = consts.tile([1, P], f32)
            nc.vector.memset(ones1, 1.0)

            ww_sb = consts.tile([FOUT, FIN], f16)
            nc.gpsimd.dma_start(out=ww_sb, in_=w_w)
            wb_sb = consts.tile([1, FOUT], f32)
            nc.gpsimd.dma_start(out=wb_sb, in_=w_b)
            wb_col = consts.tile([FOUT, 1], f32)
            nc.gpsimd.dma_start(out=wb_col, in_=w_b.rearrange("o f -> f o"))
            a1_sb = consts.tile([FOUT, 1], f32)
            nc.gpsimd.dma_start(out=a1_sb, in_=a1_d)
            a2_sb = consts.tile([1, FOUT], f32)
            nc.gpsimd.dma_start(out=a2_sb, in_=a2_d)
            ab_sb = consts.tile([1, 1], f32)
            nc.gpsimd.dma_start(out=ab_sb, in_=ab_d)

            # W_b repeated 8x along free (for the batched h_hat bias add)
            wb_rep = consts.tile([1, 8 * FOUT], f32)
            for g in range(8):
                nc.scalar.copy(wb_rep[:, g * FOUT:(g + 1) * FOUT], wb_sb)

            wwt_sb = consts.tile([P, KC * FOUT], f16)     # W_w^T chunks [128k, 64f]
            a2b_sb = consts.tile([P, FOUT], f16)          # a2 bcast along partitions
            wb_bc = consts.tile([P, 8 * FOUT], f32)       # W_b bcast, 8x repeat

            with tc.tile_pool(name="ps_init", bufs=2, space="PSUM") as ps_init:
                ps_w = ps_init.tile([P, KC * FOUT], f16, tag="w")
                for kc in range(KC):
                    nc.tensor.transpose(
                        ps_w[:, kc * FOUT:(kc + 1) * FOUT],
                        ww_sb[:, kc * P:(kc + 1) * P],
                        ident16[0:FOUT, 0:FOUT],
                    )
                nc.vector.tensor_copy(wwt_sb, ps_w)

                ps_a2 = ps_init.tile([P, FOUT], f32, tag="a2")
                nc.tensor.matmul(ps_a2, lhsT=ones1, rhs=a2_sb, start=True, stop=True)
                nc.vector.tensor_copy(a2b_sb, ps_a2)

                ps_wb = ps_init.tile([P, 8 * FOUT], f32, tag="wb")
                nc.tensor.matmul(ps_wb, lhsT=ones1, rhs=wb_rep, start=True, stop=True)
                nc.vector.tensor_copy(wb_bc, ps_wb)

            # ---- h_hat for all N nodes (j on partitions), fp16, with ones col
            hh = consts.tile([P, NJT, FOUT + 1], f16)
            nc.gpsimd.memset(hh[:, :, FOUT:FOUT + 1], 1.0)

            with (
                tc.tile_pool(name="hload", bufs=3) as hload,
                tc.tile_pool(name="hT", bufs=3) as h_t_pool,
                tc.tile_pool(name="ps_T", bufs=2, space="PSUM") as ps_t_pool,
                tc.tile_pool(name="ps_hh", bufs=2, space="PSUM") as ps_hh_pool,
            ):
                ps_hh = None
                for ht in range(NHT if stage >= 2 else 0):
                    h_t = hload.tile([P, FIN], f16, tag="h")
                    nc.gpsimd.dma_start(out=h_t, in_=h_full[hs(ht) * P:(hs(ht) + 1) * P, :])
                    ps_ht = ps_t_pool.tile([P, FIN], f16, tag="t")
                    for kc in range(KC):
                        nc.tensor.transpose(
                            ps_ht[:, kc * P:(kc + 1) * P],
                            h_t[:, kc * P:(kc + 1) * P],
                            ident16,
                        )
                    ht_sb = h_t_pool.tile([P, FIN], f16, tag="ht")
                    # alternate the PSUM->SBUF copies between DVE and ACT
                    if ht % 2 == 0:
                        nc.vector.tensor_copy(ht_sb, ps_ht)
                    else:
                        nc.scalar.copy(ht_sb, ps_ht)

                    slot = ht % 8
                    if slot == 0:
                        ps_hh = ps_hh_pool.tile([P, 8 * FOUT], f32, tag="hh")
                    for kc in range(KC):
                        nc.tensor.matmul(
                            ps_hh[:, slot * FOUT:(slot + 1) * FOUT],
                            lhsT=ht_sb[:, kc * P:(kc + 1) * P],
                            rhs=wwt_sb[:, kc * FOUT:(kc + 1) * FOUT],
                            start=(kc == 0),
                            stop=(kc == KC - 1),
                        )
                    if slot == 7:
                        g = ht // 8
                        nc.vector.tensor_tensor(
                            out=hh[:, g * 8:(g + 1) * 8, 0:FOUT],
                            in0=ps_hh[:].rearrange("p (a b) -> p a b", b=FOUT),
                            in1=wb_bc[:].rearrange("p (a b) -> p a b", b=FOUT),
                            op=OP.add,
                        )

            # ---- s2 (per key node) and its scaled/shifted form for the ACT bias
            s2a = consts.tile([P, NJT], f32)
            s2s = consts.tile([P, NJT], f32)
            with tc.tile_pool(name="scr", bufs=1) as scr:
                if stage >= 3:
                    sc = scr.tile([P, NJT, FOUT], f16, tag="s2scr")
                    a2b_ap = a2b_sb[:]
                    a2b_rep = bass.AP(
                        tensor=a2b_ap.tensor, offset=a2b_ap.offset,
                        ap=[list(a2b_ap.ap[0]), [0, NJT], list(a2b_ap.ap[1])],
                    )
                    nc.vector.tensor_tensor(
                        out=sc, in0=hh[:, :, 0:FOUT], in1=a2b_rep, op=OP.mult
                    )
                    nc.vector.tensor_reduce(
                        out=s2a[:].rearrange("p (a o) -> p a o", o=1), in_=sc,
                        axis=mybir.AxisListType.X, op=OP.add,
                    )
            if stage >= 3:
                nc.vector.tensor_scalar(s2s, s2a, 0.2, -C_SHIFT, OP.mult, OP.add)
            else:
                nc.vector.memset(s2a, 0.0)
                nc.vector.memset(s2s, 0.0)
                nc.vector.memset(hh[:, :, 0:FOUT], 0.0)

            # ---- s1 for this core's rows (via h_hat^T slice), broadcast to s1b
            s1b = consts.tile([P, RPC], f32)
            hhatt_sb = consts.tile([FOUT, RPC], f32)
            with (
                tc.tile_pool(name="hload2", bufs=2) as hload2,
                tc.tile_pool(name="hT2", bufs=2) as h_t2_pool,
                tc.tile_pool(name="ps_T2", bufs=2, space="PSUM") as ps_t2_pool,
                tc.tile_pool(name="ps_hhT", bufs=2, space="PSUM") as ps_hht_pool,
                tc.tile_pool(name="ps_s1", bufs=1, space="PSUM") as ps_s1_pool,
            ):
                ps_hht = None
                for rt in range(NIB if stage >= 3 else 0):
                    hr_t = hload2.tile([P, FIN], f16, tag="hr")
                    nc.gpsimd.dma_start(out=hr_t, in_=h_rows[hs(rt) * P:(hs(rt) + 1) * P, :])
                    ps_htr = ps_t2_pool.tile([P, FIN], f16, tag="t2")
                    for kc in range(KC):
                        nc.tensor.transpose(
                            ps_htr[:, kc * P:(kc + 1) * P],
                            hr_t[:, kc * P:(kc + 1) * P],
                            ident16,
                        )
                    htr_sb = h_t2_pool.tile([P, FIN], f16, tag="htr")
                    nc.vector.tensor_copy(htr_sb, ps_htr)

                    slot = rt % 4
                    if slot == 0:
                        ps_hht = ps_hht_pool.tile([FOUT, 4 * P], f32, tag="hht")
                    for kc in range(KC):
                        nc.tensor.matmul(
                            ps_hht[:, slot * P:(slot + 1) * P],
                            lhsT=wwt_sb[:, kc * FOUT:(kc + 1) * FOUT],
                            rhs=htr_sb[:, kc * P:(kc + 1) * P],
                            start=(kc == 0),
                            stop=(kc == KC - 1),
                        )
                    if slot == 3:
                        g = rt // 4
                        nc.scalar.activation(
                            out=hhatt_sb[:, g * 4 * P:(g + 1) * 4 * P],
                            in_=ps_hht,
                            func=AF.Identity,
                            bias=wb_col,
                            scale=1.0,
                        )

                if stage < 3:
                    nc.vector.memset(hhatt_sb, 0.0)
                ps_s1 = ps_s1_pool.tile([1, RPC], f32, tag="s1")
                for hf in range(2):
                    nc.tensor.matmul(
                        ps_s1[:, hf * HALF:(hf + 1) * HALF],
                        lhsT=a1_sb,
                        rhs=hhatt_sb[:, hf * HALF:(hf + 1) * HALF],
                        start=True,
                        stop=True,
                    )
                s1row = consts.tile([1, RPC], f32)
                nc.vector.tensor_scalar(s1row, ps_s1, ab_sb, None, OP.add)

                ps_s1b = ps_s1_pool.tile([P, RPC], f32, tag="s1b")
                for hf in range(2):
                    nc.tensor.matmul(
                        ps_s1b[:, hf * HALF:(hf + 1) * HALF],
                        lhsT=ones1,
                        rhs=s1row[:, hf * HALF:(hf + 1) * HALF],
                        start=True,
                        stop=True,
                    )
                nc.vector.tensor_copy(s1b, ps_s1b)

            if debug:
                dbg_s1b = nc.dram_tensor("dbg_s1b", [P, RPC], f32,
                                         kind="ExternalOutput")[:]
                nc.scalar.dma_start(out=dbg_s1b, in_=s1b)
                dbg_s2a = nc.dram_tensor("dbg_s2a", [P, NJT], f32,
                                         kind="ExternalOutput")[:]
                nc.scalar.dma_start(out=dbg_s2a, in_=s2a)
                dbg_hh = nc.dram_tensor("dbg_hh", [P, NJT * (FOUT + 1)], f32,
                                        kind="ExternalOutput")[:]
                hh_f32 = consts.tile([P, NJT * (FOUT + 1)], f32)
                nc.vector.tensor_copy(hh_f32, hh[:].rearrange("p a b -> p (a b)"))
                nc.scalar.dma_start(out=dbg_hh, in_=hh_f32)

            if stage < 5:
                _dummy_out(nc, tc, out_d)
            # ---- main loop over key tiles: z^T tiles + accumulate res^T
            with (
                tc.tile_pool(name="maskp", bufs=4) as maskp,
                tc.tile_pool(name="Rp", bufs=3) as rp,
                tc.tile_pool(name="Ep", bufs=3) as ep,
                tc.tile_pool(name="zp", bufs=3) as zp,
                tc.tile_pool(name="ps_res", bufs=1, space="PSUM") as ps_res_pool,
                tc.tile_pool(name="ps_epi", bufs=2, space="PSUM") as ps_epi_pool,
                tc.tile_pool(name="epi", bufs=2) as epi,
                tc.tile_pool(name="outp", bufs=2) as outp,
            ):
                res_ps = ps_res_pool.tile([FOUT + 1, RPC], f32)

                from contextlib import nullcontext

                def rep_ctx():
                    return (tc.For_i(0, dyn_reps, 1) if dyn_reps > 1
                            else nullcontext())

                with rep_ctx():
                  for rep in range(reps if stage >= 4 else 0):
                    for jt in range(NJT):
                        # cast-DMA: int32 {0,1} -> f16, already transposed
                        m_t = maskp.tile([P, RPC], f16, tag="m")
                        nc.gpsimd.dma_start(
                            out=m_t,
                            in_=mask_t[hs(jt) * P:(hs(jt) + 1) * P, :],
                        )
                        r_t = rp.tile([P, RPC], f32, tag="r")
                        nc.vector.tensor_scalar(
                            r_t, s1b, s2a[:, jt:jt + 1], 0.0, OP.add, OP.max
                        )
                        e_t = ep.tile([P, RPC], f16, tag="e")
                        nc.scalar.activation(
                            out=e_t, in_=r_t, func=AF.Exp,
                            bias=s2s[:, jt:jt + 1], scale=0.8,
                        )
                        z_t = zp.tile([P, RPC], f16, tag="z")
                        nc.vector.tensor_tensor(
                            out=z_t, in0=e_t, in1=m_t, op=OP.mult
                        )
                        if debug and rep == 0 and jt == 0:
                            dbg_z = nc.dram_tensor("dbg_z", [P, RPC], f16,
                                                   kind="ExternalOutput")[:]
                            nc.scalar.dma_start(out=dbg_z, in_=z_t)
                        for hf in range(2):
                            nc.tensor.matmul(
                                res_ps[:, hf * HALF:(hf + 1) * HALF],
                                lhsT=hh[:, jt, :],
                                rhs=z_t[:, hf * HALF:(hf + 1) * HALF],
                                start=(jt == 0),
                                stop=(jt == NJT - 1),
                            )

                    # ---- epilogue: transpose res^T back, normalize, ELU, store
                    res_sb = epi.tile([FOUT + 1, RPC], f32, tag="res")
                    nc.vector.tensor_copy(res_sb, res_ps)
                    for ib in range(NIB if stage >= 5 else 0):
                        ps_t = ps_epi_pool.tile([P, FOUT + 1], f32, tag="pst")
                        nc.tensor.transpose(
                            ps_t,
                            res_sb[:, ib * P:(ib + 1) * P],
                            ident[0:FOUT + 1, 0:FOUT + 1],
                        )
                        r_sb = epi.tile([P, 1], f32, tag="recip")
                        nc.vector.reciprocal(r_sb, ps_t[:, FOUT:FOUT + 1])
                        o_sb = epi.tile([P, FOUT], f32, tag="o")
                        nc.vector.tensor_scalar(
                            o_sb, ps_t[:, 0:FOUT], r_sb, None, OP.mult
                        )
                        xm = epi.tile([P, FOUT], f32, tag="xm")
                        nc.vector.tensor_scalar_min(xm, o_sb, 0.0)
                        eu = epi.tile([P, FOUT], f32, tag="eu")
                        nc.scalar.activation(out=eu, in_=xm, func=AF.Exp)
                        fin = outp.tile([P, FOUT], f32, tag="fin")
                        nc.vector.scalar_tensor_tensor(
                            out=fin, in0=eu, scalar=-1.0, in1=o_sb,
                            op0=OP.add, op1=OP.max,
                        )
                        nc.scalar.dma_start(
                            out=out_d[ib * P:(ib + 1) * P, :], in_=fin
                        )
    nc.finalize()
    return nc


_NC_CACHE: dict[int, bass.Bass] = {}


def _get_nc(reps: int = 1) -> bass.Bass:
    if reps not in _NC_CACHE:
        _NC_CACHE[reps] = build_nc(reps)
    return _NC_CACHE[reps]


def make_in_maps(h, attn_mask, W_w, W_b, a_w, a_b):
    h = np.ascontiguousarray(np.asarray(h, dtype=np.float32))
    attn_mask = np.ascontiguousarray(np.asarray(attn_mask, dtype=np.int32))
    W_w = np.ascontiguousarray(np.asarray(W_w, dtype=np.float32))
    W_b = np.ascontiguousarray(np.asarray(W_b, dtype=np.float32))
    a_w = np.ascontiguousarray(np.asarray(a_w, dtype=np.float32))
    a_b = np.ascontiguousarray(np.asarray(a_b, dtype=np.float32))

    # Feed each core its row-block of the attention matrix as a transposed
    # (key-major) int32 layout — a sharding/layout choice; the kernel still
    # streams the full int32 row-block from HBM.
    mask_T = attn_mask.T                     # [N keys, N queries] view
    wb_row = W_b.reshape(1, FOUT)
    a1_col = np.ascontiguousarray(a_w[0, :FOUT].reshape(FOUT, 1))
    a2_row = np.ascontiguousarray(a_w[:, FOUT:])
    ab_s = a_b.reshape(1, 1)

    in_maps = []
    for c in range(CORES):
        rows = slice(c * RPC, (c + 1) * RPC)
        in_maps.append({
            "h_full": h,
            "h_rows": h[rows],
            "maskT_rows": np.ascontiguousarray(mask_T[:, rows]),
            "W_w": W_w,
            "W_b_row": wb_row,
            "a1_col": a1_col,
            "a2_row": a2_row,
            "a_b_s": ab_s,
        })
    return in_maps


def kernel(h, attn_mask, W_w, W_b, a_w, a_b):
    nc = _get_nc()
    in_maps = make_in_maps(h, attn_mask, W_w, W_b, a_w, a_b)
    results = run_bass_kernel_spmd(nc, in_maps, list(range(CORES))).results
    out = np.concatenate([r["out_rows"] for r in results], axis=0)
    return out.astype(np.float32)


if __name__ == "__main__":
    nc = build_nc()
    print("built OK; instructions:",
          sum(len(bb.instructions) for bb in nc.m.functions[0].blocks))



# revision 16
# speedup vs baseline: 4.8529x; 4.8529x over previous
"""GAT layer (nn_GAT_layer) Trainium2 Bass kernel — 8-core SPMD, row-sharded.

Strategy (per core c of 8, query rows R_c = c*1024 .. (c+1)*1024):
  Math rewrite: with x_ij = s1_i + s2_j + a_b and leaky(x) = 0.2x + 0.8*relu(x),
    exp(leaky(x)) = exp(0.2*(s1_i+a_b)) * exp(0.2*s2_j) * exp(0.8*relu(x_ij))
  The first factor is row-constant and cancels in the softmax; the second is
  key-constant and is folded into the matmul lhs (hh_k = [hhat_j ; 1]*k_j).
  Using exp(0.8*relu(x)) = max(exp(0.8*x), 1) = max(A_i*B_j, 1) with
    A_i = exp(0.8*(s1_i + a_b)),  B_j = exp(0.8*s2_j)
  the per-tile weight needs NO transcendental in the main loop:
    z_ij = m_ij * max(T*A_i*B_j, T)   (T = 1/8 global scale, cancels)
  Main loop per key tile jt: one SWDGE uint8->fp16 casting mask DMA (batched
  4 tiles per instruction; HWDGE descriptors do not pipeline on this
  deployment, SWDGE does), one 4x-mode DVE tensor_scalar for
  u = max((T*A)*B_j, T), the mask multiply z = u*m split column-wise between
  Pool and DVE, and two accumulating PE matmuls with stationary
  [hhat_j*k_j | k_j] producing res^T[65, 1024] in PSUM.

Self-contained: hardcodes shapes from the problem spec; no sibling imports.
"""

import os
import sys

import numpy as np

for _p in ("/opt/trn_rl_repo", "/root/.axon_site/_ro/trn_rl_repo"):
    if os.path.isdir(_p) and _p not in sys.path:
        sys.path.insert(0, _p)

import concourse.bass as bass
import concourse.bacc as bacc
import concourse.tile as tile
from concourse import mybir
from concourse.masks import make_identity
from concourse.bass_utils import run_bass_kernel_spmd

N, FIN, FOUT, CORES = 8192, 256, 64, 8
P = 128
RPC = N // CORES            # 1024 query rows per core
NJT = N // P                # 64 key tiles (j on partitions)
NHT = N // P                # 64 h row-tiles
NIB = RPC // P              # 8 output row-blocks per core
KC = FIN // P               # 2 contraction chunks for h_hat
HALF = RPC // 2             # 512: matmul moving-dim max
TBATCH = int(os.environ.get("GAT_TBATCH", "4"))   # key tiles per mask DMA
T_MASK = 0.125              # clamp floor (power of two; cancels in softmax)
LN_T = -2.0794415416798357  # ln(T_MASK)
# z-multiply columns handled by the Pool engine
POOL_COLS = int(os.environ.get("GAT_POOL_COLS", "192"))
# mask in DRAM: uint8 + casting DMA (default), or fp16 + plain DMA
MASK_F16 = os.environ.get("GAT_MASK_F16", "0") == "1"

f32 = mybir.dt.float32
f16 = mybir.dt.float16
i32 = mybir.dt.int32
u8 = mybir.dt.uint8
AF = mybir.ActivationFunctionType
OP = mybir.AluOpType


def build_nc(reps: int = 1, dyn_reps: int = 0) -> bass.Bass:
    """dyn_reps>1 wraps the main loop (incl. epilogue) in a hardware For_i
    loop that repeats it dyn_reps times — identical per-iteration work, for
    wall-clock marginal timing at constant compile cost."""
    nc = bacc.Bacc(None)

    h_full = nc.dram_tensor("h_full", [N, FIN], f32, kind="ExternalInput")[:]
    h_rows = nc.dram_tensor("h_rows", [RPC, FIN], f32, kind="ExternalInput")[:]
    mask_t = nc.dram_tensor("maskT_rows", [N, RPC], f16 if MASK_F16 else u8,
                            kind="ExternalInput")[:]
    w_w = nc.dram_tensor("W_w", [FOUT, FIN], f32, kind="ExternalInput")[:]
    w_b = nc.dram_tensor("W_b_row", [1, FOUT], f32, kind="ExternalInput")[:]
    a1_d = nc.dram_tensor("a1_col", [FOUT, 1], f32, kind="ExternalInput")[:]
    a2_d = nc.dram_tensor("a2_row", [1, FOUT], f32, kind="ExternalInput")[:]
    ab_d = nc.dram_tensor("a_b_s", [1, 1], f32, kind="ExternalInput")[:]
    out_d = nc.dram_tensor("out_rows", [RPC, FOUT], f32, kind="ExternalOutput")[:]

    with tile.TileContext(nc) as tc:
        with tc.tile_pool(name="consts", bufs=1) as consts:
            ident = consts.tile([P, P], f32)
            make_identity(nc, ident)
            ident16 = consts.tile([P, P], f16)
            make_identity(nc, ident16)
            ones1 = consts.tile([1, P], f32)
            nc.vector.memset(ones1, 1.0)

            ww_sb = consts.tile([FOUT, FIN], f16)
            nc.gpsimd.dma_start(out=ww_sb, in_=w_w)
            wb_sb = consts.tile([1, FOUT], f32)
            nc.gpsimd.dma_start(out=wb_sb, in_=w_b)
            wb_col = consts.tile([FOUT, 1], f32)
            nc.gpsimd.dma_start(out=wb_col, in_=w_b.rearrange("o f -> f o"))
            a1_sb = consts.tile([FOUT, 1], f32)
            nc.gpsimd.dma_start(out=a1_sb, in_=a1_d)
            a2_sb = consts.tile([1, FOUT], f32)
            nc.gpsimd.dma_start(out=a2_sb, in_=a2_d)
            ab_sb = consts.tile([1, 1], f32)
            nc.gpsimd.dma_start(out=ab_sb, in_=ab_d)

            # W_b repeated 8x along free (for the batched h_hat bias add)
            wb_rep = consts.tile([1, 8 * FOUT], f32)
            for g in range(8):
                nc.scalar.copy(wb_rep[:, g * FOUT:(g + 1) * FOUT], wb_sb)

            wwt_sb = consts.tile([P, KC * FOUT], f16)     # W_w^T chunks [128k, 64f]
            a2b_sb = consts.tile([P, FOUT], f16)          # a2 bcast along partitions
            wb_bc = consts.tile([P, 8 * FOUT], f32)       # W_b bcast, 8x repeat

            with tc.tile_pool(name="ps_init", bufs=2, space="PSUM") as ps_init:
                ps_w = ps_init.tile([P, KC * FOUT], f16, tag="w")
                for kc in range(KC):
                    nc.tensor.transpose(
                        ps_w[:, kc * FOUT:(kc + 1) * FOUT],
                        ww_sb[:, kc * P:(kc + 1) * P],
                        ident16[0:FOUT, 0:FOUT],
                    )
                nc.vector.tensor_copy(wwt_sb, ps_w)

                ps_a2 = ps_init.tile([P, FOUT], f32, tag="a2")
                nc.tensor.matmul(ps_a2, lhsT=ones1, rhs=a2_sb, start=True, stop=True)
                nc.vector.tensor_copy(a2b_sb, ps_a2)

                ps_wb = ps_init.tile([P, 8 * FOUT], f32, tag="wb")
                nc.tensor.matmul(ps_wb, lhsT=ones1, rhs=wb_rep, start=True, stop=True)
                nc.vector.tensor_copy(wb_bc, ps_wb)

            # ---- h_hat for all N nodes (j on partitions), fp16, with ones col
            hh = consts.tile([P, NJT, FOUT + 1], f16)
            nc.gpsimd.memset(hh[:, :, FOUT:FOUT + 1], 1.0)

            with (
                tc.tile_pool(name="hload", bufs=3) as hload,
                tc.tile_pool(name="hT", bufs=3) as h_t_pool,
                tc.tile_pool(name="ps_T", bufs=2, space="PSUM") as ps_t_pool,
                tc.tile_pool(name="ps_hh", bufs=2, space="PSUM") as ps_hh_pool,
            ):
                ps_hh = None
                for ht in range(NHT):
                    h_t = hload.tile([P, FIN], f16, tag="h")
                    nc.gpsimd.dma_start(out=h_t, in_=h_full[ht * P:(ht + 1) * P, :])
                    ps_ht = ps_t_pool.tile([P, FIN], f16, tag="t")
                    for kc in range(KC):
                        nc.tensor.transpose(
                            ps_ht[:, kc * P:(kc + 1) * P],
                            h_t[:, kc * P:(kc + 1) * P],
                            ident16,
                        )
                    ht_sb = h_t_pool.tile([P, FIN], f16, tag="ht")
                    # alternate the PSUM->SBUF copies between DVE and ACT
                    if ht % 2 == 0:
                        nc.vector.tensor_copy(ht_sb, ps_ht)
                    else:
                        nc.scalar.copy(ht_sb, ps_ht)

                    slot = ht % 8
                    if slot == 0:
                        ps_hh = ps_hh_pool.tile([P, 8 * FOUT], f32, tag="hh")
                    for kc in range(KC):
                        nc.tensor.matmul(
                            ps_hh[:, slot * FOUT:(slot + 1) * FOUT],
                            lhsT=ht_sb[:, kc * P:(kc + 1) * P],
                            rhs=wwt_sb[:, kc * FOUT:(kc + 1) * FOUT],
                            start=(kc == 0),
                            stop=(kc == KC - 1),
                        )
                    if slot == 7:
                        g = ht // 8
                        nc.vector.tensor_tensor(
                            out=hh[:, g * 8:(g + 1) * 8, 0:FOUT],
                            in0=ps_hh[:].rearrange("p (a b) -> p a b", b=FOUT),
                            in1=wb_bc[:].rearrange("p (a b) -> p a b", b=FOUT),
                            op=OP.add,
                        )

            # ---- s2 (per key node): B_j = exp(0.8*s2_j), k_j = exp(0.2*s2_j)
            s2a = consts.tile([P, NJT], f32)
            bcol = consts.tile([P, NJT], f32)
            kcol = consts.tile([P, NJT], f32)
            with tc.tile_pool(name="scr", bufs=1) as scr:
                sc = scr.tile([P, NJT, FOUT], f16, tag="s2scr")
                a2b_ap = a2b_sb[:]
                a2b_rep = bass.AP(
                    tensor=a2b_ap.tensor, offset=a2b_ap.offset,
                    ap=[list(a2b_ap.ap[0]), [0, NJT], list(a2b_ap.ap[1])],
                )
                nc.vector.tensor_tensor(
                    out=sc, in0=hh[:, :, 0:FOUT], in1=a2b_rep, op=OP.mult
                )
                nc.vector.tensor_reduce(
                    out=s2a[:].rearrange("p (a o) -> p a o", o=1), in_=sc,
                    axis=mybir.AxisListType.X, op=OP.add,
                )
            nc.scalar.activation(out=bcol, in_=s2a, func=AF.Exp, scale=0.8)
            nc.scalar.activation(out=kcol, in_=s2a, func=AF.Exp, scale=0.2)

            # fold k_j into the matmul lhs: hh[:, j, :] *= k_j (incl. ones col)
            kc_ap = kcol[:]
            kc_rep = bass.AP(
                tensor=kc_ap.tensor, offset=kc_ap.offset,
                ap=[list(kc_ap.ap[0]), list(kc_ap.ap[1]), [0, FOUT + 1]],
            )
            nc.vector.tensor_tensor(out=hh, in0=hh, in1=kc_rep, op=OP.mult)

            # ---- s1 for this core's rows (via h_hat^T slice) -> A row, bcast
            hhatt_sb = consts.tile([FOUT, RPC], f32)
            with (
                tc.tile_pool(name="hload2", bufs=2) as hload2,
                tc.tile_pool(name="hT2", bufs=2) as h_t2_pool,
                tc.tile_pool(name="ps_T2", bufs=2, space="PSUM") as ps_t2_pool,
                tc.tile_pool(name="ps_hhT", bufs=2, space="PSUM") as ps_hht_pool,
                tc.tile_pool(name="ps_s1", bufs=1, space="PSUM") as ps_s1_pool,
            ):
                ps_hht = None
                for rt in range(NIB):
                    hr_t = hload2.tile([P, FIN], f16, tag="hr")
                    nc.gpsimd.dma_start(out=hr_t, in_=h_rows[rt * P:(rt + 1) * P, :])
                    ps_htr = ps_t2_pool.tile([P, FIN], f16, tag="t2")
                    for kc in range(KC):
                        nc.tensor.transpose(
                            ps_htr[:, kc * P:(kc + 1) * P],
                            hr_t[:, kc * P:(kc + 1) * P],
                            ident16,
                        )
                    htr_sb = h_t2_pool.tile([P, FIN], f16, tag="htr")
                    nc.vector.tensor_copy(htr_sb, ps_htr)

                    slot = rt % 4
                    if slot == 0:
                        ps_hht = ps_hht_pool.tile([FOUT, 4 * P], f32, tag="hht")
                    for kc in range(KC):
                        nc.tensor.matmul(
                            ps_hht[:, slot * P:(slot + 1) * P],
                            lhsT=wwt_sb[:, kc * FOUT:(kc + 1) * FOUT],
                            rhs=htr_sb[:, kc * P:(kc + 1) * P],
                            start=(kc == 0),
                            stop=(kc == KC - 1),
                        )
                    if slot == 3:
                        g = rt // 4
                        nc.scalar.activation(
                            out=hhatt_sb[:, g * 4 * P:(g + 1) * 4 * P],
                            in_=ps_hht,
                            func=AF.Identity,
                            bias=wb_col,
                            scale=1.0,
                        )

                ps_s1 = ps_s1_pool.tile([1, RPC], f32, tag="s1")
                for hf in range(2):
                    nc.tensor.matmul(
                        ps_s1[:, hf * HALF:(hf + 1) * HALF],
                        lhsT=a1_sb,
                        rhs=hhatt_sb[:, hf * HALF:(hf + 1) * HALF],
                        start=True,
                        stop=True,
                    )
                s1row = consts.tile([1, RPC], f32)
                nc.vector.tensor_scalar(s1row, ps_s1, ab_sb, None, OP.add)

                # A_i = exp(0.8*(s1_i + a_b)), broadcast to all partitions, f16
                ab_f16 = consts.tile([P, RPC], f16)
                s1b = consts.tile([P, RPC], f32)
                ps_s1b = ps_s1_pool.tile([P, RPC], f32, tag="s1b")
                for hf in range(2):
                    nc.tensor.matmul(
                        ps_s1b[:, hf * HALF:(hf + 1) * HALF],
                        lhsT=ones1,
                        rhs=s1row[:, hf * HALF:(hf + 1) * HALF],
                        start=True,
                        stop=True,
                    )
                nc.vector.tensor_copy(s1b, ps_s1b)
                lnt_col = consts.tile([P, 1], f32)
                nc.vector.memset(lnt_col, LN_T)
                nc.scalar.activation(out=ab_f16, in_=s1b, func=AF.Exp,
                                     scale=0.8, bias=lnt_col)

            # ---- main loop over key tiles: z^T tiles + accumulate res^T
            with (
                tc.tile_pool(name="maskp", bufs=3) as maskp,
                tc.tile_pool(name="tp", bufs=3) as tp,
                tc.tile_pool(name="zp", bufs=3) as zp,
                tc.tile_pool(name="ps_res", bufs=1, space="PSUM") as ps_res_pool,
                tc.tile_pool(name="ps_epi", bufs=2, space="PSUM") as ps_epi_pool,
                tc.tile_pool(name="epi", bufs=2) as epi,
                tc.tile_pool(name="outp", bufs=2) as outp,
            ):
                res_ps = ps_res_pool.tile([FOUT + 1, RPC], f32)

                from contextlib import nullcontext

                rep_ctx = (tc.For_i(0, dyn_reps, 1) if dyn_reps > 1
                           else nullcontext())
                with rep_ctx:
                  for rep in range(reps):
                    for jb in range(NJT // TBATCH):
                        m4 = maskp.tile([P, TBATCH, RPC], f16, tag="m")
                        nc.gpsimd.dma_start(
                            out=m4,
                            in_=mask_t[jb * TBATCH * P:(jb + 1) * TBATCH * P, :]
                            .rearrange("(t p) i -> p t i", p=P),
                        )
                        for tt in range(TBATCH):
                            jt = jb * TBATCH + tt
                            m_t = m4[:, tt, :]
                            # u = max(A_i*B_j, T) * T  (4x-mode tensor_scalar)
                            u_t = tp.tile([P, RPC], f16, tag="t")
                            nc.vector.tensor_scalar(
                                u_t, ab_f16, bcol[:, jt:jt + 1], T_MASK,
                                OP.mult, OP.max,
                            )
                            # z = u * m, columns split between Pool and DVE
                            z_t = zp.tile([P, RPC], f16, tag="z")
                            if POOL_COLS:
                                nc.gpsimd.tensor_tensor(
                                    out=z_t[:, 0:POOL_COLS],
                                    in0=u_t[:, 0:POOL_COLS],
                                    in1=m_t[:, 0:POOL_COLS], op=OP.mult,
                                )
                            nc.vector.tensor_tensor(
                                out=z_t[:, POOL_COLS:],
                                in0=u_t[:, POOL_COLS:],
                                in1=m_t[:, POOL_COLS:], op=OP.mult,
                            )
                            for hf in range(2):
                                nc.tensor.matmul(
                                    res_ps[:, hf * HALF:(hf + 1) * HALF],
                                    lhsT=hh[:, jt, :],
                                    rhs=z_t[:, hf * HALF:(hf + 1) * HALF],
                                    start=(jt == 0),
                                    stop=(jt == NJT - 1),
                                )

                    # ---- epilogue: transpose res^T back, normalize, ELU, store
                    res_sb = epi.tile([FOUT + 1, RPC], f32, tag="res")
                    nc.vector.tensor_copy(res_sb, res_ps)
                    for ib in range(NIB):
                        ps_t = ps_epi_pool.tile([P, FOUT + 1], f32, tag="pst")
                        nc.tensor.transpose(
                            ps_t,
                            res_sb[:, ib * P:(ib + 1) * P],
                            ident[0:FOUT + 1, 0:FOUT + 1],
                        )
                        r_sb = epi.tile([P, 1], f32, tag="recip")
                        nc.vector.reciprocal(r_sb, ps_t[:, FOUT:FOUT + 1])
                        o_sb = epi.tile([P, FOUT], f32, tag="o")
                        nc.vector.tensor_scalar(
                            o_sb, ps_t[:, 0:FOUT], r_sb, None, OP.mult
                        )
                        xm = epi.tile([P, FOUT], f32, tag="xm")
                        nc.vector.tensor_scalar_min(xm, o_sb, 0.0)
                        eu = epi.tile([P, FOUT], f32, tag="eu")
                        nc.scalar.activation(out=eu, in_=xm, func=AF.Exp)
                        fin = outp.tile([P, FOUT], f32, tag="fin")
                        nc.vector.scalar_tensor_tensor(
                            out=fin, in0=eu, scalar=-1.0, in1=o_sb,
                            op0=OP.add, op1=OP.max,
                        )
                        nc.scalar.dma_start(
                            out=out_d[ib * P:(ib + 1) * P, :], in_=fin
                        )
    nc.finalize()
    return nc


_NC_CACHE: dict[int, bass.Bass] = {}


def _get_nc(reps: int = 1) -> bass.Bass:
    if reps not in _NC_CACHE:
        _NC_CACHE[reps] = build_nc(reps)
    return _NC_CACHE[reps]


def make_in_maps(h, attn_mask, W_w, W_b, a_w, a_b):
    h = np.ascontiguousarray(np.asarray(h, dtype=np.float32))
    attn_mask = np.asarray(attn_mask)
    W_w = np.ascontiguousarray(np.asarray(W_w, dtype=np.float32))
    W_b = np.ascontiguousarray(np.asarray(W_b, dtype=np.float32))
    a_w = np.ascontiguousarray(np.asarray(a_w, dtype=np.float32))
    a_b = np.ascontiguousarray(np.asarray(a_b, dtype=np.float32))

    # Key-major (transposed) mask with values {0, 1}; each core gets its
    # column block (its query rows). uint8 flavor is cast to f16 by the DMA.
    mask_T = (attn_mask.T != 0).astype(np.float16 if MASK_F16 else np.uint8)
    wb_row = W_b.reshape(1, FOUT)
    a1_col = np.ascontiguousarray(a_w[0, :FOUT].reshape(FOUT, 1))
    a2_row = np.ascontiguousarray(a_w[:, FOUT:])
    ab_s = a_b.reshape(1, 1)

    in_maps = []
    for c in range(CORES):
        rows = slice(c * RPC, (c + 1) * RPC)
        in_maps.append({
            "h_full": h,
            "h_rows": h[rows],
            "maskT_rows": np.ascontiguousarray(mask_T[:, rows]),
            "W_w": W_w,
            "W_b_row": wb_row,
            "a1_col": a1_col,
            "a2_row": a2_row,
            "a_b_s": ab_s,
        })
    return in_maps


def kernel(h, attn_mask, W_w, W_b, a_w, a_b):
    nc = _get_nc()
    in_maps = make_in_maps(h, attn_mask, W_w, W_b, a_w, a_b)
    results = run_bass_kernel_spmd(nc, in_maps, list(range(CORES))).results
    out = np.concatenate([r["out_rows"] for r in results], axis=0)
    return out.astype(np.float32)


if __name__ == "__main__":
    nc = build_nc()
    print("built OK; instructions:",
          sum(len(bb.instructions) for bb in nc.m.functions[0].blocks))
